# revision 20
# baseline (speedup 1.0000x reference)
"""GammaScorer edge-scoring kernel for 8 Trainium2 NeuronCores.

score[e] = sigmoid((x[src[e]] * x[dst[e]]) @ W.T + b)

Strategy: shard the 640K edges across 8 cores; replicate the node table x
and the tiny linear weights on every core. Row gathers use the batched
SWDGE dma_gather instruction (int16 indices, <=1024 rows per instruction
due to the 16KB descriptor-ring carveout), which amortizes the ~1us
fixed descriptor-generation cost per DMA instruction that dominated the
naive per-column indirect-DMA approach.

dma_gather indices are int16, so the 100K-row table is addressed in 4
chunks of 25K rows. Host-side, edges are bucketed globally by
(src_chunk, dst_chunk) into 16 buckets and each bucket is split evenly
across the 8 cores, so every core runs an identical static program
(SPMD) with per-bucket sizes known at compile time. Scores come back in
bucket-major order and are unpermuted on the host.

Per 1024-edge unit: two dma_gathers (src rows, dst rows) land as
[128, cols, 128] tiles (edge j -> partition j%128, column j//128); DVE
computes u = s*t then a fused tensor_tensor_reduce per column does
(u * W) summed over features; Act applies sigmoid(+bias).
"""

import sys

import numpy as np

sys.path.insert(0, "/opt/trn_rl_repo")

N_NODES = 100000
D = 128
E = 640000
N_CORES = 8
P = 128
NCHUNKS = 4
CHUNK = N_NODES // NCHUNKS  # 25000 rows per chunk: fits int16 indices
UNIT = 1024                 # max rows per dma_gather (SWDGE ring capacity)
NB = NCHUNKS * NCHUNKS      # 16 (src_chunk, dst_chunk) buckets

_CACHE = {}


def _plan(src_idx, dst_idx):
    """Bucket edges globally by (src chunk, dst chunk), split each bucket
    evenly over the 8 cores, and precompute gather-unit geometry shared by
    all cores plus the metadata to unpermute the scores."""
    src = np.asarray(src_idx).astype(np.int64)
    dst = np.asarray(dst_idx).astype(np.int64)
    bucket = (src // CHUNK) * NCHUNKS + dst // CHUNK
    order = np.argsort(bucket, kind="stable")
    counts = np.bincount(bucket, minlength=NB)

    # per-core bucket sizes, multiples of 16 (idx-stream wrap granularity).
    # Remainder units leave stale data in the tail of their last column; the
    # kernel pre-zeroes the gather buffers once so those slots stay finite.
    m = (-(-counts // N_CORES) + 15) // 16 * 16  # [NB]
    M = int(m.sum())  # padded edges per core

    eid = np.full((N_CORES, M), -1, dtype=np.int64)
    src16 = np.zeros((N_CORES, M), dtype=np.int16)
    dst16 = np.zeros((N_CORES, M), dtype=np.int16)
    starts = np.concatenate([[0], np.cumsum(counts)])
    off = 0
    for b in range(NB):
        ids_b = order[starts[b] : starts[b + 1]]
        padded = np.full(N_CORES * m[b], -1, np.int64)
        padded[: len(ids_b)] = ids_b
        padded = padded.reshape(N_CORES, m[b])
        eid[:, off : off + m[b]] = padded
        valid = padded >= 0
        sa, db = b // NCHUNKS, b % NCHUNKS
        s_loc = np.zeros(padded.shape, dtype=np.int64)
        d_loc = np.zeros(padded.shape, dtype=np.int64)
        s_loc[valid] = src[padded[valid]] - sa * CHUNK
        d_loc[valid] = dst[padded[valid]] - db * CHUNK
        src16[:, off : off + m[b]] = s_loc.astype(np.int16)
        dst16[:, off : off + m[b]] = d_loc.astype(np.int16)
        off += m[b]

    # gather units: (src_chunk, dst_chunk, idx_col0, nidx, col0, cols)
    units = []
    col0 = 0
    off = 0
    for b in range(NB):
        sa, db = b // NCHUNKS, b % NCHUNKS
        for j0 in range(0, int(m[b]), UNIT):
            nidx = int(min(UNIT, m[b] - j0))
            cols = -(-nidx // 128)
            units.append((sa, db, (off + j0) // 16, nidx, col0, cols))
            col0 += cols
        off += int(m[b])
    totc = col0

    # stream position j -> (partition, column) in the [P, totc] score tile
    prow = np.empty(M, np.int64)
    pcol = np.empty(M, np.int64)
    jof = 0
    for b in range(NB):
        for j0 in range(0, int(m[b]), UNIT):
            pass  # geometry handled below via units
    for sa, db, ic0, nidx, c0, cols in units:
        j0 = ic0 * 16  # stream offset of this unit
        i = np.arange(nidx)
        prow[j0 : j0 + nidx] = i % 128
        pcol[j0 : j0 + nidx] = c0 + i // 128
        jof += nidx
    assert jof == M

    return {
        "m": m,
        "M": M,
        "units": units,
        "totc": totc,
        "eid": eid,
        "src16": src16,
        "dst16": dst16,
        "prow": prow,
        "pcol": pcol,
    }


def _wrap16(stream):
    """[M] int16 -> [128, M/16] with stream[j] at [j%16, j//16], replicated
    across the 8 groups of 16 partitions (one per descriptor-gen core)."""
    M = stream.shape[0]
    base = stream.reshape(M // 16, 16).T  # [16, M/16]
    return np.ascontiguousarray(np.tile(base, (8, 1)))


def _build_nc(units, M, totc):
    from contextlib import ExitStack

    import concourse.bacc as bacc
    import concourse.tile as tile
    from concourse import mybir

    f32 = mybir.dt.float32
    bf16 = mybir.dt.bfloat16
    i16 = mybir.dt.int16
    UC = UNIT // 128  # columns per full unit (8)

    nc = bacc.Bacc(
        "TRN2",
        target_bir_lowering=False,
        debug=False,
        num_devices=N_CORES,
    )
    x = nc.dram_tensor("x", [N_NODES, D], f32, kind="ExternalInput")
    srcw = nc.dram_tensor("srcw", [P, M // 16], i16, kind="ExternalInput")
    dstw = nc.dram_tensor("dstw", [P, M // 16], i16, kind="ExternalInput")
    wrep = nc.dram_tensor("wrep", [P, UC * D], bf16, kind="ExternalInput")
    brep = nc.dram_tensor("brep", [P, 1], f32, kind="ExternalInput")
    out = nc.dram_tensor("out", [P, totc], f32, kind="ExternalOutput")

    BUFS = 8
    with tile.TileContext(nc) as tc, ExitStack() as ctx:
        const = ctx.enter_context(tc.tile_pool(name="const", bufs=1))
        work = ctx.enter_context(tc.tile_pool(name="work", bufs=BUFS))
        res = ctx.enter_context(tc.tile_pool(name="res", bufs=1))

        # chunked idx loads so the first gathers start before the whole
        # stream has landed
        NCH = 8
        icols = M // 16
        srcw_sb = const.tile([P, icols], i16)
        dstw_sb = const.tile([P, icols], i16)
        bnd = [icols * i // NCH for i in range(NCH + 1)]
        for i in range(NCH):
            nc.sync.dma_start(srcw_sb[:, bnd[i] : bnd[i + 1]], srcw[:, bnd[i] : bnd[i + 1]])
            nc.sync.dma_start(dstw_sb[:, bnd[i] : bnd[i + 1]], dstw[:, bnd[i] : bnd[i + 1]])
        w_sb = const.tile([P, UC * D], bf16)
        nc.sync.dma_start(w_sb[:], wrep[:])
        b_sb = const.tile([P, 1], f32)
        nc.sync.dma_start(b_sb[:], brep[:])

        scores = res.tile([P, totc], f32)

        for sa, db, ic0, nidx, c0, cols in units:
            s_t = work.tile([P, UC * D], f32, tag="S")
            t_t = work.tile([P, UC * D], f32, tag="T")
            # remainder units leave the tail of the last column unwritten;
            # pre-zero that column (Act engine) so downstream reads stay
            # finite — the gather then overwrites the valid slots
            if nidx % 128:
                nc.scalar.memzero(s_t[:, (cols - 1) * D : cols * D])
                nc.scalar.memzero(t_t[:, (cols - 1) * D : cols * D])
            icn = -(-nidx // 16)
            nc.gpsimd.dma_gather(
                out_ap=s_t[:, : cols * D].rearrange("p (c d) -> p c d", d=D),
                in_ap=x[sa * CHUNK : (sa + 1) * CHUNK, :],
                idxs_ap=srcw_sb[:, ic0 : ic0 + icn],
                num_idxs=nidx,
                num_idxs_reg=nidx,
                elem_size=D,
            )
            nc.gpsimd.dma_gather(
                out_ap=t_t[:, : cols * D].rearrange("p (c d) -> p c d", d=D),
                in_ap=x[db * CHUNK : (db + 1) * CHUNK, :],
                idxs_ap=dstw_sb[:, ic0 : ic0 + icn],
                num_idxs=nidx,
                num_idxs_reg=nidx,
                elem_size=D,
            )
            u_t = work.tile([P, UC * D], bf16, tag="U")
            nc.vector.tensor_mul(
                u_t[:, : cols * D], s_t[:, : cols * D], t_t[:, : cols * D]
            )
            v_t = work.tile([P, UC * D], bf16, tag="V")
            nc.vector.tensor_mul(
                v_t[:, : cols * D], u_t[:, : cols * D], w_sb[:, : cols * D]
            )
            # partial tree reduction in bf16 (2x DVE mode), then f32 reduce
            v3 = v_t[:, : cols * D].rearrange("p (k d) -> p k d", d=D)
            h1 = work.tile([P, UC * D // 2], bf16, tag="H1")
            h1_3 = h1[:, : cols * D // 2].rearrange("p (k d) -> p k d", d=D // 2)
            nc.vector.tensor_add(h1_3, v3[:, :, 0 : D // 2], v3[:, :, D // 2 : D])
            h2 = work.tile([P, UC * D // 4], bf16, tag="H2")
            h2_3 = h2[:, : cols * D // 4].rearrange("p (k d) -> p k d", d=D // 4)
            nc.vector.tensor_add(h2_3, h1_3[:, :, 0 : D // 4], h1_3[:, :, D // 4 :])
            dots = work.tile([P, UC], f32, tag="dots")
            nc.vector.reduce_sum(dots[:, :cols], h2_3, axis=mybir.AxisListType.X)
            nc.scalar.activation(
                scores[:, c0 : c0 + cols],
                dots[:, :cols],
                mybir.ActivationFunctionType.Sigmoid,
                bias=b_sb[:],
            )

        # chunked stores: each depends only on the activations covering it,
        # so earlier chunks stream out while later units still compute
        sb = [totc * i // NCH for i in range(NCH + 1)]
        for i in range(NCH):
            nc.sync.dma_start(out[:, sb[i] : sb[i + 1]], scores[:, sb[i] : sb[i + 1]])

    nc.compile()
    return nc


def kernel(x, src_idx, dst_idx, W, b):
    from concourse.bass_utils import run_bass_kernel_spmd

    x = np.ascontiguousarray(np.asarray(x), dtype=np.float32)
    W = np.asarray(W, dtype=np.float32)
    b = np.asarray(b, dtype=np.float32)

    plan = _plan(src_idx, dst_idx)

    key = (plan["M"], plan["totc"], tuple(int(v) for v in plan["m"]))
    if key not in _CACHE:
        _CACHE[key] = _build_nc(plan["units"], plan["M"], plan["totc"])
    nc = _CACHE[key]
    _CACHE["last_nc"] = nc

    import ml_dtypes

    wrep = np.ascontiguousarray(
        np.tile(W.reshape(1, D), (P, UNIT // 128)).astype(ml_dtypes.bfloat16)
    )
    brep = np.full((P, 1), b.reshape(-1)[0], dtype=np.float32)

    in_maps = []
    for c in range(N_CORES):
        in_maps.append(
            {
                "x": x,
                "srcw": _wrap16(plan["src16"][c]),
                "dstw": _wrap16(plan["dst16"][c]),
                "wrep": wrep,
                "brep": brep,
            }
        )

    results = run_bass_kernel_spmd(nc, in_maps, list(range(N_CORES))).results

    out_full = np.empty(E, dtype=np.float32)
    prow, pcol = plan["prow"], plan["pcol"]
    for c in range(N_CORES):
        stream = np.asarray(results[c]["out"])[prow, pcol]
        mask = plan["eid"][c] >= 0
        out_full[plan["eid"][c][mask]] = stream[mask]
    return out_full.reshape(E, 1).astype(np.float32)


# revision 35
# speedup vs baseline: 1.0082x; 1.0082x over previous
"""GammaScorer edge-scoring kernel for 8 Trainium2 NeuronCores.

score[e] = sigmoid((x[src[e]] * x[dst[e]]) @ W.T + b)

Strategy: shard the 640K edges across 8 cores; replicate the node table x
and the tiny linear weights on every core. Row gathers use the batched
SWDGE dma_gather instruction (int16 indices, <=1024 rows per instruction
due to the 16KB descriptor-ring carveout), which amortizes the ~1us
fixed descriptor-generation cost per DMA instruction that dominated the
naive per-column indirect-DMA approach.

dma_gather indices are int16, so the 100K-row table is addressed in 4
chunks of 25K rows. Host-side, edges are bucketed globally by
(src_chunk, dst_chunk) into 16 buckets and each bucket is split evenly
across the 8 cores, so every core runs an identical static program
(SPMD) with per-bucket sizes known at compile time. Scores come back in
bucket-major order and are unpermuted on the host.

Per 1024-edge unit: two dma_gathers (src rows, dst rows) land as
[128, cols, 128] tiles (edge j -> partition j%128, column j//128); DVE
computes u = s*t then a fused tensor_tensor_reduce per column does
(u * W) summed over features; Act applies sigmoid(+bias).
"""

import sys

import numpy as np

sys.path.insert(0, "/opt/trn_rl_repo")

N_NODES = 100000
D = 128
E = 640000
N_CORES = 8
P = 128
NCHUNKS = 4
CHUNK = N_NODES // NCHUNKS  # 25000 rows per chunk: fits int16 indices
UNIT = 1024                 # max rows per dma_gather (SWDGE ring capacity)
NB = NCHUNKS * NCHUNKS      # 16 (src_chunk, dst_chunk) buckets

_CACHE = {}


def _plan(src_idx, dst_idx):
    """Bucket edges globally by (src chunk, dst chunk), split each bucket
    evenly over the 8 cores, and precompute gather-unit geometry shared by
    all cores plus the metadata to unpermute the scores."""
    src = np.asarray(src_idx).astype(np.int64)
    dst = np.asarray(dst_idx).astype(np.int64)
    bucket = (src // CHUNK) * NCHUNKS + dst // CHUNK
    order = np.argsort(bucket, kind="stable")
    counts = np.bincount(bucket, minlength=NB)

    # per-core bucket sizes, multiples of 16 (idx-stream wrap granularity).
    # Remainder units leave stale data in the tail of their last column; the
    # kernel pre-zeroes the gather buffers once so those slots stay finite.
    m = (-(-counts // N_CORES) + 15) // 16 * 16  # [NB]
    M = int(m.sum())  # padded edges per core

    eid = np.full((N_CORES, M), -1, dtype=np.int64)
    src16 = np.zeros((N_CORES, M), dtype=np.int16)
    dst16 = np.zeros((N_CORES, M), dtype=np.int16)
    starts = np.concatenate([[0], np.cumsum(counts)])
    off = 0
    for b in range(NB):
        ids_b = order[starts[b] : starts[b + 1]]
        padded = np.full(N_CORES * m[b], -1, np.int64)
        padded[: len(ids_b)] = ids_b
        padded = padded.reshape(N_CORES, m[b])
        eid[:, off : off + m[b]] = padded
        valid = padded >= 0
        sa, db = b // NCHUNKS, b % NCHUNKS
        s_loc = np.zeros(padded.shape, dtype=np.int64)
        d_loc = np.zeros(padded.shape, dtype=np.int64)
        s_loc[valid] = src[padded[valid]] - sa * CHUNK
        d_loc[valid] = dst[padded[valid]] - db * CHUNK
        src16[:, off : off + m[b]] = s_loc.astype(np.int16)
        dst16[:, off : off + m[b]] = d_loc.astype(np.int16)
        off += m[b]

    # gather units: (src_chunk, dst_chunk, idx_col0, nidx, col0, cols)
    units = []
    col0 = 0
    off = 0
    for b in range(NB):
        sa, db = b // NCHUNKS, b % NCHUNKS
        for j0 in range(0, int(m[b]), UNIT):
            nidx = int(min(UNIT, m[b] - j0))
            cols = -(-nidx // 128)
            units.append((sa, db, (off + j0) // 16, nidx, col0, cols))
            col0 += cols
        off += int(m[b])
    totc = col0

    # stream position j -> (partition, column) in the [P, totc] score tile
    prow = np.empty(M, np.int64)
    pcol = np.empty(M, np.int64)
    jof = 0
    for b in range(NB):
        for j0 in range(0, int(m[b]), UNIT):
            pass  # geometry handled below via units
    for sa, db, ic0, nidx, c0, cols in units:
        j0 = ic0 * 16  # stream offset of this unit
        i = np.arange(nidx)
        prow[j0 : j0 + nidx] = i % 128
        pcol[j0 : j0 + nidx] = c0 + i // 128
        jof += nidx
    assert jof == M

    return {
        "m": m,
        "M": M,
        "units": units,
        "totc": totc,
        "eid": eid,
        "src16": src16,
        "dst16": dst16,
        "prow": prow,
        "pcol": pcol,
    }


def _wrap16(stream):
    """[M] int16 -> [128, M/16] with stream[j] at [j%16, j//16], replicated
    across the 8 groups of 16 partitions (one per descriptor-gen core)."""
    M = stream.shape[0]
    base = stream.reshape(M // 16, 16).T  # [16, M/16]
    return np.ascontiguousarray(np.tile(base, (8, 1)))


def _build_nc(units, M, totc):
    from contextlib import ExitStack

    import concourse.bacc as bacc
    import concourse.tile as tile
    from concourse import mybir

    f32 = mybir.dt.float32
    bf16 = mybir.dt.bfloat16
    i16 = mybir.dt.int16
    UC = UNIT // 128  # columns per full unit (8)

    nc = bacc.Bacc(
        "TRN2",
        target_bir_lowering=False,
        debug=False,
        num_devices=N_CORES,
    )
    x = nc.dram_tensor("x", [N_NODES, D], f32, kind="ExternalInput")
    srcw = nc.dram_tensor("srcw", [P, M // 16], i16, kind="ExternalInput")
    dstw = nc.dram_tensor("dstw", [P, M // 16], i16, kind="ExternalInput")
    wrep = nc.dram_tensor("wrep", [P, UC * D], bf16, kind="ExternalInput")
    brep = nc.dram_tensor("brep", [P, 1], f32, kind="ExternalInput")
    out = nc.dram_tensor("out", [P, totc], f32, kind="ExternalOutput")

    BUFS = 8
    with tile.TileContext(nc) as tc, ExitStack() as ctx:
        const = ctx.enter_context(tc.tile_pool(name="const", bufs=1))
        work = ctx.enter_context(tc.tile_pool(name="work", bufs=BUFS))
        res = ctx.enter_context(tc.tile_pool(name="res", bufs=1))

        # chunked idx loads so the first gathers start before the whole
        # stream has landed
        NCH = 4
        icols = M // 16
        srcw_sb = const.tile([P, icols], i16)
        dstw_sb = const.tile([P, icols], i16)
        bnd = [icols * i // NCH for i in range(NCH + 1)]
        for i in range(NCH):
            nc.sync.dma_start(srcw_sb[:, bnd[i] : bnd[i + 1]], srcw[:, bnd[i] : bnd[i + 1]])
            nc.sync.dma_start(dstw_sb[:, bnd[i] : bnd[i + 1]], dstw[:, bnd[i] : bnd[i + 1]])
        w_sb = const.tile([P, UC * D], bf16)
        nc.sync.dma_start(w_sb[:], wrep[:])
        b_sb = const.tile([P, 1], f32)
        nc.sync.dma_start(b_sb[:], brep[:])

        scores = res.tile([P, totc], f32)

        for sa, db, ic0, nidx, c0, cols in units:
            s_t = work.tile([P, UC * D], f32, tag="S")
            t_t = work.tile([P, UC * D], f32, tag="T")
            if nidx % 128:
                nc.scalar.memzero(s_t[:, (cols - 1) * D : cols * D])
                nc.scalar.memzero(t_t[:, (cols - 1) * D : cols * D])
            icn = -(-nidx // 16)
            nc.gpsimd.dma_gather(
                out_ap=s_t[:, : cols * D].rearrange("p (c d) -> p c d", d=D),
                in_ap=x[sa * CHUNK : (sa + 1) * CHUNK, :],
                idxs_ap=srcw_sb[:, ic0 : ic0 + icn],
                num_idxs=nidx,
                num_idxs_reg=nidx,
                elem_size=D,
            )
            nc.gpsimd.dma_gather(
                out_ap=t_t[:, : cols * D].rearrange("p (c d) -> p c d", d=D),
                in_ap=x[db * CHUNK : (db + 1) * CHUNK, :],
                idxs_ap=dstw_sb[:, ic0 : ic0 + icn],
                num_idxs=nidx,
                num_idxs_reg=nidx,
                elem_size=D,
            )
            u_t = work.tile([P, UC * D], bf16, tag="U")
            nc.vector.tensor_mul(
                u_t[:, : cols * D], s_t[:, : cols * D], t_t[:, : cols * D]
            )
            v_t = work.tile([P, UC * D], bf16, tag="V")
            nc.vector.tensor_mul(
                v_t[:, : cols * D], u_t[:, : cols * D], w_sb[:, : cols * D]
            )
            # one bf16 2x-mode halving pass, then f32 reduce over 64
            v3 = v_t[:, : cols * D].rearrange("p (k d) -> p k d", d=D)
            h1 = work.tile([P, UC * D // 2], bf16, tag="H1")
            h1_3 = h1[:, : cols * D // 2].rearrange("p (k d) -> p k d", d=D // 2)
            nc.vector.tensor_add(h1_3, v3[:, :, 0 : D // 2], v3[:, :, D // 2 : D])
            dots = work.tile([P, UC], f32, tag="dots")
            nc.vector.reduce_sum(dots[:, :cols], h1_3, axis=mybir.AxisListType.X)
            nc.scalar.activation(
                scores[:, c0 : c0 + cols],
                dots[:, :cols],
                mybir.ActivationFunctionType.Sigmoid,
                bias=b_sb[:],
            )

        # chunked stores: each depends only on the activations covering it,
        # so earlier chunks stream out while later units still compute
        sb = [totc * i // NCH for i in range(NCH + 1)]
        for i in range(NCH):
            nc.sync.dma_start(out[:, sb[i] : sb[i + 1]], scores[:, sb[i] : sb[i + 1]])

    nc.compile()
    return nc


def kernel(x, src_idx, dst_idx, W, b):
    from concourse.bass_utils import run_bass_kernel_spmd

    x = np.ascontiguousarray(np.asarray(x), dtype=np.float32)
    W = np.asarray(W, dtype=np.float32)
    b = np.asarray(b, dtype=np.float32)

    plan = _plan(src_idx, dst_idx)

    key = (plan["M"], plan["totc"], tuple(int(v) for v in plan["m"]))
    if key not in _CACHE:
        _CACHE[key] = _build_nc(plan["units"], plan["M"], plan["totc"])
    nc = _CACHE[key]
    _CACHE["last_nc"] = nc

    import ml_dtypes

    wrep = np.ascontiguousarray(
        np.tile(W.reshape(1, D), (P, UNIT // 128)).astype(ml_dtypes.bfloat16)
    )
    brep = np.full((P, 1), b.reshape(-1)[0], dtype=np.float32)

    in_maps = []
    for c in range(N_CORES):
        in_maps.append(
            {
                "x": x,
                "srcw": _wrap16(plan["src16"][c]),
                "dstw": _wrap16(plan["dst16"][c]),
                "wrep": wrep,
                "brep": brep,
            }
        )

    results = run_bass_kernel_spmd(nc, in_maps, list(range(N_CORES))).results

    out_full = np.empty(E, dtype=np.float32)
    prow, pcol = plan["prow"], plan["pcol"]
    for c in range(N_CORES):
        stream = np.asarray(results[c]["out"])[prow, pcol]
        mask = plan["eid"][c] >= 0
        out_full[plan["eid"][c][mask]] = stream[mask]
    return out_full.reshape(E, 1).astype(np.float32)
